# revision 49
# baseline (speedup 1.0000x reference)
"""Gemma3 sliding-window attention on 8 Trainium2 NeuronCores.

Sharding: core c handles batch b=c//4 and head-group g=c%4 (4 of 16 q heads,
2 of 8 kv heads). wq/wk/wv column-split, wo row-split; the 4 partial outputs
per batch are summed on host (no device collectives).

Design (v3, 222.1us TimelineSim, rel err 1.4e-3):
- QKV projections in fp8(e4m3) hi/lo pairs via DoubleRow matmuls (the three
  (hi,hi),(hi,lo),(lo,hi) cross terms at 0.5 cyc/row) -- the PE-optimal mix
  that still meets the 2e-2 error gate; everything 2-byte is fp16 (identical
  PE/DVE cost to bf16, ~3.5x lower end-to-end error than bf16).
- exp biased by -2 (cancels in the softmax divide) keeps probs inside fp16
  range; sliding-window masks are 0/1 fp16 multiplies on the idle GPSIMD.
- The DMA transfer resource is one serial ~360GB/s FIFO: all input loads
  ride the SP queue in exact consumption order (hs/wv split into first-use
  pieces), rot partition-shift DMAs split across SP/Act HWDGE queues.
- Engine placement: Act = proj-copy + sqrt in phase 1, pure exp in phase 2
  (one act-table switch, preloaded by a dummy exp before the transition);
  DVE = square/recip/rope muls, softmax divides, y copies; GPSIMD = rope
  sin-muls and mask muls (it cannot touch PSUM).
- Phase 2 runs (u0, kvh) units through a LAG-2 scores->exp->PV pipeline
  (6-deep priming at the phase boundary), with per-(slab, kv-head)
  transpose triggering, tiny ramp units interleaved 2:1 into the big-unit
  stream to smooth the Act exp load (the last four drain the tail), and a
  ~10-chain outproj reserve so the PE never starves while exps trickle in.
- Pitfalls baked in: matmul accumulation chains must stay contiguous
  (interleaved open chains corrupt PSUM); tile pools only order against
  already-emitted readers, so pool wrap-around must trail chain emission.
"""

import math
import os
import numpy as np
import ml_dtypes

_DEF = {"K_PRIMES": 4, "K_TINY": 1, "K_LAGP": 3, "K_HOLD": 10,
        "K_MASKMM": 0, "K_LAG": 2, "K_TSIN": 1}
_K = lambda name, d=None: int(os.environ.get(name, _DEF.get(name, d)))

import concourse.bacc as bacc
import concourse.mybir as mybir
import concourse.tile as tile
from concourse.bass_utils import run_bass_kernel_spmd

dt = mybir.dt
AFT = mybir.ActivationFunctionType
ALU = mybir.AluOpType
BF = dt.bfloat16
FP16 = dt.float16
F32 = dt.float32

B, S, H = 2, 2048, 2048
NQC, NKVC, D = 4, 2, 128          # per-core heads
WIN = 1024
EPS = 1e-6
THETA = 10000.0
P = 128
SCP = 512                          # phase-1 seq chunk
NCH = S // SCP                     # 4
NHT = H // P                       # 16
NST = S // P                       # 16
WT = WIN // P                      # 8 (window in tiles)
LAG = None                         # set below from K_LAG
EXP_BIAS = -2.0                    # exp(s-2): fp16-safe probs; cancels in div

_CACHE = {}


def _build_nc():
    if "nc" in _CACHE:
        return _CACHE["nc"]
    nc = bacc.Bacc("TRN2", target_bir_lowering=False, debug=False, num_devices=8)

    F8 = dt.float8e4
    DR = mybir.MatmulPerfMode.DoubleRow
    # hi/lo fp8 pairs: x ~= hi + lo to ~0.1% rms; DoubleRow matmuls run the
    # (hi,hi), (hi,lo), (lo,hi) cross terms at 0.5 cyc/row over ht-pairs.
    hs_d = nc.dram_tensor("hs", [P, NCH, 2, NHT // 2, 2, 4, P], F8,
                          kind="ExternalInput").ap()
    wq_d = nc.dram_tensor("wq", [P, 2, NHT // 2, 2, NQC * D], F8,
                          kind="ExternalInput").ap()
    wk_d = nc.dram_tensor("wk", [P, 2, NHT // 2, 2, NKVC * D], F8,
                          kind="ExternalInput").ap()
    wv_d = nc.dram_tensor("wv", [P, 2, NHT // 2, 2, NKVC * D], F8,
                          kind="ExternalInput").ap()
    wo_d = nc.dram_tensor("wo", [P, NQC, H], FP16, kind="ExternalInput").ap()
    tabs_d = nc.dram_tensor("tabs", [P, 4, S], FP16, kind="ExternalInput").ap()
    msk_d = nc.dram_tensor("msk", [P, 9, P], FP16, kind="ExternalInput").ap()
    y_d = nc.dram_tensor("y", [P, NHT, S], FP16, kind="ExternalOutput").ap()

    with nc.allow_low_precision(reason="fp16/fp8 kernel; rel-err budget 2e-2"), \
         tile.TileContext(nc) as tc:
        with (
            tc.tile_pool(name="const", bufs=1) as cpool,
            tc.tile_pool(name="qkv", bufs=1) as qkv,
            tc.tile_pool(name="wts", bufs=1) as wts,
        ):
            msk_sb = cpool.tile([P, 9, P], FP16, tag="msk")
            ones_sb = cpool.tile([P, P], BF, tag="ones")
            eps_sb = cpool.tile([P, 1], F32, tag="eps")
            eb_sb = cpool.tile([P, 1], F32, tag="eb")
            nc.vector.memset(ones_sb[:], 1.0)
            nc.vector.memset(eps_sb[:], EPS)
            nc.vector.memset(eb_sb[:], EXP_BIAS)
            id_sb = msk_sb[:, 0, :]
            dm_sb = msk_sb[:, 1:3, :]
            em_sb = msk_sb[:, 3:5, :]
            dm01_sb = msk_sb[:, 5:7, :]
            em01_sb = msk_sb[:, 7:9, :]

            wv_sb = wts.tile([P, 2, NHT // 2, 2, NKVC * D], F8, tag="wv")
            wk_sb = wts.tile([P, 2, NHT // 2, 2, NKVC * D], F8, tag="wk")
            wq_sb = wts.tile([P, 2, NHT // 2, 2, NQC * D], F8, tag="wq")
            wo_sb = wts.tile([P, NQC, H], FP16, tag="wo")

            qn_sb = qkv.tile([P, NQC, S], FP16, tag="qn")
            kn_sb = qkv.tile([P, NKVC, S], FP16, tag="kn")
            # x16-scaled fp8 copies, partition-folded (d -> [64, 2] pairs
            # (2p, 2p+1)) for DoubleRow scores on the off-diagonal tiles
            qn8_sb = qkv.tile([64, 2, NQC, S], F8, tag="qn8")
            kn8_sb = qkv.tile([64, 2, NKVC, S], F8, tag="kn8")
            v_sb = qkv.tile([P, NST, NKVC, D + 1], FP16, tag="v")
            nc.vector.memset(v_sb[:, :, :, D:D + 1], 64.0)

            # ---------------- phase 1: QKV projections + RMSNorm + RoPE ----
            with (
                tc.tile_pool(name="hsp", bufs=3) as hsp,
                tc.tile_pool(name="tabp", bufs=3) as tabp,
                tc.tile_pool(name="cpp", bufs=7) as cpp,
                tc.tile_pool(name="t1", bufs=8) as t1p,
                tc.tile_pool(name="t2", bufs=3) as t2p,
                tc.tile_pool(name="t3", bufs=3) as t3p,
                tc.tile_pool(name="t5", bufs=5) as t5p,
                tc.tile_pool(name="t6", bufs=5) as t6p,
                tc.tile_pool(name="pp", bufs=_K("K_PPP", 4), space="PSUM") as ppp,
                tc.tile_pool(name="pvb", bufs=_K("K_PVB", 2), space="PSUM") as pvbp,
                tc.tile_pool(name="psv", bufs=2, space="PSUM") as psvp,
            ):
                # heads order: k0, k1, q0..q3 (wk arrives before wq)
                HEADS = [("k", 0), ("k", 1), ("q", 0), ("q", 1), ("q", 2), ("q", 3)]
                pend = []  # deferred norm/rope finishes (2-deep pipeline)

                def proj_chain(out_ps, w_sb8, mcols, hs_t):
                    first = True
                    for wi, xi in ((0, 0), (0, 1), (1, 0)):
                        for tp in range(NHT // 2):
                            nc.tensor.matmul(
                                out_ps[:], w_sb8[:, wi, tp, :, mcols],
                                hs_t[:, xi, tp, :, :, :],
                                perf_mode=DR, start=first,
                                stop=(wi == 1 and tp == NHT // 2 - 1))
                            first = False

                def finishA(st, tsin_eng=None):
                    # early stage: Pool tsin + DVE square + PE sumsq matmul.
                    # Decoupled from the Pool-gated add so the next head's
                    # sq/vb never queue behind a tsin round-trip.
                    kind, m, pp, cp, u_t, s0, tab_t, rot = st
                    tsin = t1p.tile([P, SCP], FP16, tag="tsin")
                    sin_t = tab_t[:, 1 if kind == "q" else 3, :]
                    eng = nc.gpsimd if _K("K_TSIN", 1) else nc.vector
                    if _K("K_TSINQ3", 0) and s0 == (NCH - 1) * SCP and \
                            kind == "q" and m >= 4 - _K("K_TSINQ3", 0):
                        # last-chunk q tail: Pool's serial tsin chain is the
                        # transition critical path -> run these on DVE where
                        # the in-order queue sequences them before their adds
                        eng = nc.vector
                    (tsin_eng or eng).tensor_mul(tsin[:], rot[:], sin_t)
                    sq = t1p.tile([P, SCP], BF, tag="sq")
                    nc.vector.tensor_mul(sq[:], cp[:], cp[:])
                    vb = pvbp.tile([P, SCP], F32, tag="vb")
                    nc.tensor.matmul(vb[:], ones_sb[:], sq[:], start=True, stop=True)
                    return (kind, m, u_t, s0, tsin, vb)

                def finishB(stB):
                    kind, m, u_t, s0, tsin, vb = stB
                    sd = t2p.tile([P, SCP], FP16, tag="sd")
                    nc.scalar.activation(sd[:], vb[:], AFT.Sqrt, bias=eps_sb[:],
                                         scale=1.0 / D)
                    inv = t3p.tile([P, SCP], FP16, tag="inv")
                    nc.vector.reciprocal(inv[:], sd[:])
                    nc.vector.tensor_add(u_t[:], u_t[:], tsin[:])
                    dst = qn_sb if kind == "q" else kn_sb
                    nc.vector.tensor_mul(dst[:, m, s0:s0 + SCP], u_t[:], inv[:])
                    # fp8 copy + partition-fold for the DR scores path
                    q8u = t2p.tile([P, SCP], F8, tag="q8u")
                    nc.vector.tensor_copy(q8u[:], dst[:, m, s0:s0 + SCP])
                    dst8 = qn8_sb if kind == "q" else kn8_sb
                    nc.scalar.dma_start(out=dst8[:, :, m, s0:s0 + SCP],
                                        in_=q8u[:])

                pend2 = []

                def finish(st, tsin_eng=None):
                    pend2.append(finishA(st, tsin_eng))
                    while len(pend2) > _K("K_PEND2", 1):
                        finishB(pend2.pop(0))

                chunk_tiles = {}

                def load_chunk(sc):
                    s0 = sc * SCP
                    hs_sb = hsp.tile([P, 2, NHT // 2, 2, 4, P], F8, tag="hs",
                                     name="hs_sb")
                    nc.sync.dma_start(out=hs_sb[:, 0], in_=hs_d[:, sc, 0])
                    nc.sync.dma_start(out=hs_sb[:, 1], in_=hs_d[:, sc, 1])
                    tab_sb = tabp.tile([P, 4, SCP], FP16, tag="tab",
                                       name="tab_sb")
                    nc.sync.dma_start(out=tab_sb[:],
                                      in_=tabs_d[:, :, s0:s0 + SCP])
                    chunk_tiles[sc] = (hs_sb, tab_sb)

                for sc in range(NCH):
                    s0 = sc * SCP
                    if sc > 0:
                        hs_sb, tab_sb = chunk_tiles[sc]
                    else:
                        hs_sb = hsp.tile([P, 2, NHT // 2, 2, 4, P], F8,
                                         tag="hs", name="hs_sb")
                        tab_sb = tabp.tile([P, 4, SCP], FP16, tag="tab",
                                           name="tab_sb")
                        # startup: the DMA transfer resource is one serial
                        # FIFO, so everything rides the SP queue in exact
                        # consumption order; hs halves split so the v chain
                        # starts on the first pieces.
                        nc.sync.dma_start(out=wv_sb[:, 0, 0:2], in_=wv_d[:, 0, 0:2])
                        nc.sync.dma_start(out=hs_sb[:, 0, 0:2], in_=hs_d[:, 0, 0, 0:2])
                        nc.sync.dma_start(out=wv_sb[:, 0, 2:8], in_=wv_d[:, 0, 2:8])
                        nc.sync.dma_start(out=hs_sb[:, 0, 2:4], in_=hs_d[:, 0, 0, 2:4])
                        nc.sync.dma_start(out=hs_sb[:, 0, 4:8], in_=hs_d[:, 0, 0, 4:8])
                        nc.sync.dma_start(out=wv_sb[:, 1], in_=wv_d[:, 1])
                        nc.sync.dma_start(out=hs_sb[:, 1, 0:4], in_=hs_d[:, 0, 1, 0:4])
                        nc.sync.dma_start(out=hs_sb[:, 1, 4:8], in_=hs_d[:, 0, 1, 4:8])
                        nc.sync.dma_start(out=wk_sb[:, 0], in_=wk_d[:, 0])
                        nc.sync.dma_start(out=wk_sb[:, 1], in_=wk_d[:, 1])
                        nc.sync.dma_start(out=tab_sb[:], in_=tabs_d[:, :, 0:SCP])
                        nc.sync.dma_start(out=wq_sb[:, 0], in_=wq_d[:, 0])
                        nc.sync.dma_start(out=wq_sb[:, 1], in_=wq_d[:, 1])
                        nc.sync.dma_start(out=msk_sb[:], in_=msk_d[:])

                    # v projection, 2 ss per PSUM tile; chunk 0 runs the three
                    # hi/lo passes ss-interleaved so each input piece is
                    # needed as late as possible.
                    pv_tiles = []
                    for sp in range(2):
                        pv = psvp.tile([P, 2, NKVC * D], F32, tag="pv")
                        pv_tiles.append(pv)
                    # NOTE: matmul accumulation chains must stay contiguous
                    # (interleaving open chains within a PSUM bank corrupts
                    # the accumulation) -> per-ss chains; chunk 0 orders its
                    # passes (hi,hi),(lo-w,hi),(hi,lo-hs) so the late-arriving
                    # hs-lo piece is consumed last.
                    passes = ((0, 0), (1, 0), (0, 1)) if sc == 0 else \
                             ((0, 0), (0, 1), (1, 0))
                    if False:
                        pass
                    else:
                        for ss in range(4):
                            first = True
                            for wi, xi in passes:
                                for tp in range(NHT // 2):
                                    nc.tensor.matmul(
                                        pv_tiles[ss // 2][:, ss % 2, :],
                                        hs_sb[:, xi, tp, :, ss, :],
                                        wv_sb[:, wi, tp, :, :],
                                        perf_mode=DR, start=first,
                                        stop=(wi == 1 and tp == NHT // 2 - 1))
                                    first = False
                    for ss in range(4):
                        st_g = sc * 4 + ss
                        nc.vector.tensor_copy(v_sb[:, st_g, :, 0:D],
                                              pv_tiles[ss // 2][:, ss % 2, :])
                        if ss == 0 and pend:
                            finish(pend.pop(0))
                        if ss == 0 and sc + 1 < NCH:
                            load_chunk(sc + 1)
                        if ss == 0 and sc == 2:
                            nc.sync.dma_start(out=wo_sb[:], in_=wo_d[:])

                    for kind, m in HEADS:
                        w_sb = wq_sb if kind == "q" else wk_sb
                        pp = ppp.tile([P, SCP], F32, tag="pp")
                        proj_chain(pp, w_sb, slice(m * D, (m + 1) * D), hs_sb)
                        cp = cpp.tile([P, SCP], FP16, tag="cp")
                        if _K("K_CPTAIL", 1) and sc == NCH - 1 and \
                                kind == "q" and m >= 2:
                            nc.vector.tensor_copy(cp[:], pp[:])
                        else:
                            nc.scalar.copy(cp[:], pp[:])
                        # rotate_half as a partition-shift DMA (sign and norm
                        # weight are folded into the sin tables on the host);
                        # the two halves ride different HWDGE queues.
                        rot = t6p.tile([P, SCP], FP16, tag="rot")
                        nc.sync.dma_start(out=rot[0:64, :], in_=cp[64:128, :])
                        nc.scalar.dma_start(out=rot[64:128, :], in_=cp[0:64, :])
                        u_t = t5p.tile([P, SCP], FP16, tag="u")
                        cos_t = tab_sb[:, 0 if kind == "q" else 2, :]
                        nc.vector.tensor_mul(u_t[:], cp[:], cos_t)
                        pend.append((kind, m, pp, cp, u_t, s0, tab_sb, rot))
                        depth = 1 if (sc == NCH - 1 and kind == "q" and m >= 1) else 2
                        while len(pend) > depth:
                            finish(pend.pop(0))
                for st in pend:
                    finish(st, tsin_eng=(nc.vector if _K("K_TSINTAIL", 0)
                                         else None))
                while pend2:
                    finishB(pend2.pop(0))
                # preload the Exp act-table set before the attention phase
                dmy = t2p.tile([P, 1], FP16, tag="dmy")
                nc.scalar.activation(dmy[:], eps_sb[:], AFT.Exp)

            # ---------------- phase 2+3: attention + output projection -----
            with (
                tc.tile_pool(name="pb", bufs=_K("K_LAG", 2) + _K("K_PBX", 6)) as pbp,
                tc.tile_pool(name="invp", bufs=4) as invp,
                tc.tile_pool(name="aq", bufs=2) as aqp,
                tc.tile_pool(name="aT", bufs=3) as aTp,
                tc.tile_pool(name="ysb", bufs=2) as ysp,
                tc.tile_pool(name="psc", bufs=_K("K_PSC", 3), space="PSUM") as pscp,
                tc.tile_pool(name="pa", bufs=2, space="PSUM") as pap,
                tc.tile_pool(name="psy", bufs=_K("K_PSY", 3), space="PSUM") as psyp,
            ):
                queue = []
                slab_tiles = {}

                def emit_scores(kvh, u0):
                    # paired unit: both q heads of this kv head at once
                    h0 = 2 * kvh
                    t0 = max(0, u0 - WT)
                    n = u0 - t0 + 1
                    p_t = pbp.tile([P, WT + 1, 2, P], FP16, tag="p", name="p_t")
                    qn_sl = qn_sb[:, h0:h0 + 2, u0 * P:(u0 + 1) * P]
                    qn8_sl = qn8_sb[:, :, h0:h0 + 2, u0 * P:(u0 + 1) * P]
                    for g0 in range(0, n, 2):
                        gn = min(2, n - g0)
                        sc_t = pscp.tile([P, 2, 2, P], F32, tag="sc",
                                         name="sc_t")
                        for i in range(gn):
                            t = t0 + g0 + i
                            # sliding-window mask folded into the PSUM chain
                            # as an additive -60 (exp -> exact fp16 zero)
                            madd = dm_sb if t == u0 else (
                                em_sb if (u0 >= WT and t == u0 - WT) else None)
                            use_mm = _K("K_MASKMM", 1)
                            if t == u0:
                                # diagonal tile in fp16: softmax-peaked rows
                                # live here; fp8 there costs 3x the error
                                nc.tensor.matmul(
                                    sc_t[:, i, :, :],
                                    kn_sb[:, kvh, t * P:(t + 1) * P],
                                    qn_sl, start=True,
                                    stop=(madd is None or not use_mm))
                            else:
                                nc.tensor.matmul(
                                    sc_t[:, i, :, :],
                                    kn8_sb[:, :, kvh, t * P:(t + 1) * P],
                                    qn8_sl, perf_mode=DR, start=True,
                                    stop=(madd is None or not use_mm))
                            if madd is not None and use_mm:
                                nc.tensor.matmul(
                                    sc_t[:, i, :, :], id_sb, madd,
                                    start=False, stop=True)
                        nc.scalar.activation(p_t[:, g0:g0 + gn, :, :],
                                             sc_t[:, 0:gn, :, :], AFT.Exp,
                                             bias=eb_sb[:], scale=1.0 / 256.0)
                        if not _K("K_MASKMM", 1):
                            for i in range(gn):
                                t = t0 + g0 + i
                                if t == u0:
                                    blk = p_t[:, g0 + i, :, :]
                                    nc.gpsimd.tensor_mul(blk, blk, dm01_sb)
                                elif u0 >= WT and t == u0 - WT:
                                    blk = p_t[:, g0 + i, :, :]
                                    nc.gpsimd.tensor_mul(blk, blk, em01_sb)
                    return (kvh, u0, t0, p_t)

                def emit_pv(st):
                    kvh, u0, t0, p_t = st
                    n = u0 - t0 + 1
                    slab = slab_tiles[u0 // 4]
                    for hh in range(2):
                        h = 2 * kvh + hh
                        a_t = pap.tile([P, D + 1], F32, tag="a")
                        for i in range(n):
                            nc.tensor.matmul(
                                a_t[:], p_t[:, i, hh, :],
                                v_sb[:, t0 + i, kvh, :],
                                start=(i == 0), stop=(i == n - 1))
                        inv = invp.tile([P, 1], F32, tag="inv")
                        nc.vector.reciprocal(inv[:], a_t[:, D:D + 1])
                        nc.vector.tensor_scalar_mul(slab[:, h, u0 % 4, :],
                                                    a_t[:, 0:D], inv[:])

                def emit_transpose(s4, h):
                    if ("T", s4) not in slab_tiles:
                        aT = aTp.tile([P, NQC, 4, P], FP16, tag="aT", name="aT")
                        slab_tiles[("T", s4)] = aT
                    aT = slab_tiles[("T", s4)]
                    slab = slab_tiles[s4]
                    eng = nc.scalar if h % 2 == 0 else nc.sync
                    eng.dma_start_transpose(out=aT[:, h, :, :],
                                            in_=slab[:, h, :, :])

                op_queue = []  # (s4, mo) outproj chains, spread across units
                op_state = {}
                ycount = [0]

                def emit_outproj_chain():
                    if not op_queue:
                        return
                    s4, mo = op_queue.pop(0)
                    aT = slab_tiles[("T", s4)]
                    mog, mo4 = divmod(mo, 4)
                    if mo4 == 0:
                        op_state["y"] = ysp.tile([P, 4, SCP], FP16, tag="y",
                                                 name="y_t")
                    y_t = op_state["y"]
                    yp = psyp.tile([P, SCP], F32, tag="yp")
                    for h in range(NQC):
                        nc.tensor.matmul(
                            yp[:], wo_sb[:, h, mo * P:(mo + 1) * P],
                            aT[:, h, :, :],
                            start=(h == 0), stop=(h == NQC - 1))
                    # y copies on DVE (GPSIMD can't read PSUM; Act stays a
                    # pure-exp engine until the tail, where it helps drain)
                    ycount[0] += 1
                    if op_state.get("tail") and ycount[0] % 2 == 0:
                        nc.scalar.copy(y_t[:, mo4, :], yp[:])
                    else:
                        nc.vector.tensor_copy(y_t[:, mo4, :], yp[:])
                    if s4 == 0 or _K("K_YMO", 0):
                        # per-mo output DMAs: finer serial-FIFO granularity
                        nc.sync.dma_start(
                            out=y_d[:, mo, s4 * SCP:(s4 + 1) * SCP],
                            in_=y_t[:, mo4, :])
                    elif mo4 == 3:
                        nc.sync.dma_start(
                            out=y_d[:, mog * 4:(mog + 1) * 4,
                                    s4 * SCP:(s4 + 1) * SCP],
                            in_=y_t[:])

                def emit_outproj(s4):
                    op_queue.extend((s4, mo) for mo in range(NHT))

                # descending u0: the big steady-state units come first and
                # prime the PV pipeline; the small ramp units land at the end
                # where the outproj slabs provide PE filler work.
                # Unit order: prime with kvh=0 big units (q2/q3 rope
                # finishes still in flight), then the tiny ramp slabs 0
                # (their exp latency hides behind big-unit work), then the
                # rest descending so the drain tail lands on slab 1 whose 16
                # outproj chains feed the PE while the last exps trickle in.
                pm = _K("K_PRIMES", 6)
                if pm == 6:
                    UNITS = [(15, 0), (14, 0), (13, 0),
                             (15, 1), (14, 1), (13, 1)]
                elif pm == 7:
                    UNITS = [(15, 0), (14, 0), (13, 0), (12, 0),
                             (11, 0), (10, 0)]
                elif pm == 8:
                    UNITS = [(15, 0), (14, 0), (13, 0), (12, 0),
                             (15, 1), (14, 1)]
                else:
                    UNITS = [(15, 0), (14, 0), (15, 1), (14, 1)]
                big = [(u0, kvh) for u0 in range(13, 3, -1)
                       for kvh in range(NKVC)]
                tiny = [(u0, kvh) for u0 in range(3, -1, -1)
                        for kvh in range(NKVC)]
                big = [u for u in big if u not in UNITS]
                tmode = _K("K_TINY", 1)
                bi = ti = 0
                ratio = {0: 2, 1: 4, 2: 3}[tmode]
                tsave = {0: 0, 1: 4, 2: 2}[tmode]
                while bi < len(big):
                    for _ in range(ratio):
                        if bi < len(big):
                            UNITS.append(big[bi]); bi += 1
                    if ti < len(tiny) - tsave:
                        UNITS.append(tiny[ti]); ti += 1
                UNITS.extend(tiny[ti:])
                slab_pv_left = {}
                for u0, kvh in UNITS:
                    k = (u0 // 4, kvh)
                    slab_pv_left[k] = slab_pv_left.get(k, 0) + 1
                pending_tr = []

                def note_pv_done(u0, kvh):
                    k = (u0 // 4, kvh)
                    slab_pv_left[k] -= 1
                    if slab_pv_left[k] == 0:
                        pending_tr.append(k)

                tr_done = {}

                def emit_slab_transposes(k):
                    s4, kvh = k
                    emit_transpose(s4, 2 * kvh)
                    emit_transpose(s4, 2 * kvh + 1)
                    tr_done[s4] = tr_done.get(s4, 0) + 1
                    if tr_done[s4] == 2:
                        emit_outproj(s4)

                seen_slab = set()
                ucount = [0]
                for u0, kvh in UNITS:
                    if u0 // 4 not in seen_slab:
                        seen_slab.add(u0 // 4)
                        slab_tiles[u0 // 4] = aqp.tile([P, NQC, 4, P], FP16,
                                                       tag="aq", name="aq")
                    queue.append(emit_scores(kvh, u0))
                    ucount[0] += 1
                    lag_eff = _K("K_LAGP", 6) if ucount[0] <= 8 else _K("K_LAG", 2)
                    if len(queue) > lag_eff:
                        st = queue.pop(0)
                        emit_pv(st)
                        note_pv_done(st[1], st[0])
                    if pending_tr:
                        emit_slab_transposes(pending_tr.pop(0))
                    emit_outproj_chain()
                    if len(op_queue) > _K("K_HOLD", 10):
                        emit_outproj_chain()
                # drain: PV tail with transposes interleaved
                while queue:
                    st = queue.pop(0)
                    emit_pv(st)
                    note_pv_done(st[1], st[0])
                    while pending_tr:
                        emit_slab_transposes(pending_tr.pop(0))
                op_state["tail"] = True
                while op_queue:
                    emit_outproj_chain()

    nc.compile()
    _CACHE["nc"] = nc
    return nc


def _host_inputs(hidden_states, wq, wk, wv, wo, q_norm_weight, k_norm_weight):
    """Per-core input dicts (8 cores: c = 4*b + g)."""
    f16 = np.float16
    scale = 1.0 / math.sqrt(D)
    inv_freq = 1.0 / (THETA ** (np.arange(0, D, 2, dtype=np.float64) / D))
    t = np.arange(S, dtype=np.float64)
    freqs = np.outer(t, inv_freq)
    emb = np.concatenate([freqs, freqs], axis=-1)          # [S, D]
    cosT = np.cos(emb).T.astype(np.float64)                # [D, S]
    sinT = np.sin(emb).T.astype(np.float64)
    qw = (1.0 + q_norm_weight).astype(np.float64)
    kw = (1.0 + k_norm_weight).astype(np.float64)

    # rotate_half is done on-device as a plain partition-shift DMA
    # (rot[d] = x[(d+64)%128]); the rotation sign and the SHIFTED norm
    # weight are folded into the sin tables here:
    #   sin_eff[d] = sgn(d) * sin[d] * w[(d+64)%128],  sgn = -1 for d<64
    hh = D // 2
    sgn = np.where(np.arange(D) < hh, -1.0, 1.0)
    qw_sh = np.roll(qw, -hh)   # w[(d+64)%128]
    kw_sh = np.roll(kw, -hh)
    tabs = np.stack([
        cosT * qw[:, None] * scale * 16.0,
        sinT * (sgn * qw_sh)[:, None] * scale * 16.0,
        cosT * kw[:, None] * 16.0,
        sinT * (sgn * kw_sh)[:, None] * 16.0,
    ], axis=1).astype(f16)                                 # [D, 4, S] x16:
    # qn,kn carry x16 so every score tile (fp16 diag and fp8 DR) is x256,
    # absorbed by one exp scale=1/256

    r = np.arange(P)[:, None]
    c = np.arange(P)[None, :]
    dmadd = np.where(c >= r, 0.0, -60.0)                   # [k, q]: allow q >= k
    emadd = np.where(c < r, 0.0, -60.0)                    # [k, q]: allow q < k
    ident = np.eye(P)
    dm01 = np.where(c >= r, 1.0, 0.0)
    em01 = np.where(c < r, 1.0, 0.0)
    msk = np.stack([ident, dmadd, dmadd, emadd, emadd,
                    dm01, dm01, em01, em01],
                   axis=1).astype(f16)                      # [k, 9, q]

    f8 = ml_dtypes.float8_e4m3
    WS = 64.0  # weight scale: fp8 sweet spot; absorbed by RMSNorm (q/k) and
               # by the 64-valued ones-column of v_ext (v)

    def hi_lo(x):
        hi = x.astype(f8)
        lo = (x - hi.astype(np.float64)).astype(f8)
        return np.stack([hi, lo], axis=1)

    def pack_w(w_slice):
        # [O, H] -> lhsT [H, O] -> [128, 2(hl), 8(tp), 2(ti), O] fp8 x64
        wT = w_slice.T.astype(np.float64) * WS
        O = wT.shape[1]
        base = wT.reshape(NHT // 2, 2, P, O).transpose(2, 0, 1, 3)
        return np.ascontiguousarray(hi_lo(base))

    hs_packed = []
    for b in range(B):
        hsT = hidden_states[b].T.astype(np.float64)        # [H, S]
        # [p, sc, 2(hl), 8(tp), 2(ti), 4(q), s128] fp8 hi/lo
        hs6 = hsT.reshape(NHT // 2, 2, P, NCH, 4, P).transpose(2, 3, 0, 1, 4, 5)
        hi = hs6.astype(f8)
        lo = (hs6 - hi.astype(np.float64)).astype(f8)
        hs_packed.append(np.ascontiguousarray(np.stack([hi, lo], axis=2)))

    in_maps = []
    for core in range(8):
        b, g = divmod(core, 4)
        woT = wo[:, 512 * g:512 * (g + 1)].T.astype(np.float64)  # [512, H]
        wo_r = np.ascontiguousarray(
            woT.reshape(NQC, P, H).transpose(1, 0, 2)).astype(f16)
        in_maps.append({
            "hs": hs_packed[b],
            "wq": pack_w(wq[512 * g:512 * (g + 1), :]),
            "wk": pack_w(wk[256 * g:256 * (g + 1), :]),
            "wv": pack_w(wv[256 * g:256 * (g + 1), :]),
            "wo": wo_r,
            "tabs": tabs, "msk": msk,
        })
    return in_maps


def _postprocess(results):
    out = np.empty((B, S, H), np.float32)
    for b in range(B):
        acc = np.zeros((H, S), np.float32)
        for g in range(4):
            y_r = results[4 * b + g]["y"].astype(np.float32)  # [128, 16, S]
            acc += y_r.transpose(1, 0, 2).reshape(H, S)
        out[b] = acc.T
    return out


def kernel(hidden_states, wq, wk, wv, wo, q_norm_weight, k_norm_weight):
    nc = _build_nc()
    in_maps = _host_inputs(hidden_states, wq, wk, wv, wo,
                           q_norm_weight, k_norm_weight)
    res = run_bass_kernel_spmd(nc, in_maps, list(range(8)))
    return _postprocess(res.results)


# revision 50
# speedup vs baseline: 1.0132x; 1.0132x over previous
"""Gemma3 sliding-window attention on 8 Trainium2 NeuronCores.

Sharding: core c handles batch b=c//4 and head-group g=c%4 (4 of 16 q heads,
2 of 8 kv heads). wq/wk/wv column-split, wo row-split; the 4 partial outputs
per batch are summed on host (no device collectives).

Design (v3, 222.1us TimelineSim, rel err 1.4e-3):
- QKV projections in fp8(e4m3) hi/lo pairs via DoubleRow matmuls (the three
  (hi,hi),(hi,lo),(lo,hi) cross terms at 0.5 cyc/row) -- the PE-optimal mix
  that still meets the 2e-2 error gate; everything 2-byte is fp16 (identical
  PE/DVE cost to bf16, ~3.5x lower end-to-end error than bf16).
- exp biased by -2 (cancels in the softmax divide) keeps probs inside fp16
  range; sliding-window masks are 0/1 fp16 multiplies on the idle GPSIMD.
- The DMA transfer resource is one serial ~360GB/s FIFO: all input loads
  ride the SP queue in exact consumption order (hs/wv split into first-use
  pieces), rot partition-shift DMAs split across SP/Act HWDGE queues.
- Engine placement: Act = proj-copy + sqrt in phase 1, pure exp in phase 2
  (one act-table switch, preloaded by a dummy exp before the transition);
  DVE = square/recip/rope muls, softmax divides, y copies; GPSIMD = rope
  sin-muls and mask muls (it cannot touch PSUM).
- Phase 2 runs (u0, kvh) units through a LAG-2 scores->exp->PV pipeline
  (6-deep priming at the phase boundary), with per-(slab, kv-head)
  transpose triggering, tiny ramp units interleaved 2:1 into the big-unit
  stream to smooth the Act exp load (the last four drain the tail), and a
  ~10-chain outproj reserve so the PE never starves while exps trickle in.
- Pitfalls baked in: matmul accumulation chains must stay contiguous
  (interleaved open chains corrupt PSUM); tile pools only order against
  already-emitted readers, so pool wrap-around must trail chain emission.
"""

import math
import os
import numpy as np
import ml_dtypes

_DEF = {"K_PRIMES": 4, "K_TINY": 1, "K_LAGP": 1, "K_HOLD": 11,
        "K_MASKMM": 0, "K_LAG": 2, "K_TSIN": 1}
_K = lambda name, d=None: int(os.environ.get(name, _DEF.get(name, d)))

import concourse.bacc as bacc
import concourse.mybir as mybir
import concourse.tile as tile
from concourse.bass_utils import run_bass_kernel_spmd

dt = mybir.dt
AFT = mybir.ActivationFunctionType
ALU = mybir.AluOpType
BF = dt.bfloat16
FP16 = dt.float16
F32 = dt.float32

B, S, H = 2, 2048, 2048
NQC, NKVC, D = 4, 2, 128          # per-core heads
WIN = 1024
EPS = 1e-6
THETA = 10000.0
P = 128
SCP = 512                          # phase-1 seq chunk
NCH = S // SCP                     # 4
NHT = H // P                       # 16
NST = S // P                       # 16
WT = WIN // P                      # 8 (window in tiles)
LAG = None                         # set below from K_LAG
EXP_BIAS = -2.0                    # exp(s-2): fp16-safe probs; cancels in div

_CACHE = {}


def _build_nc():
    if "nc" in _CACHE:
        return _CACHE["nc"]
    nc = bacc.Bacc("TRN2", target_bir_lowering=False, debug=False, num_devices=8)

    F8 = dt.float8e4
    DR = mybir.MatmulPerfMode.DoubleRow
    # hi/lo fp8 pairs: x ~= hi + lo to ~0.1% rms; DoubleRow matmuls run the
    # (hi,hi), (hi,lo), (lo,hi) cross terms at 0.5 cyc/row over ht-pairs.
    hs_d = nc.dram_tensor("hs", [P, NCH, 2, NHT // 2, 2, 4, P], F8,
                          kind="ExternalInput").ap()
    wq_d = nc.dram_tensor("wq", [P, 2, NHT // 2, 2, NQC * D], F8,
                          kind="ExternalInput").ap()
    wk_d = nc.dram_tensor("wk", [P, 2, NHT // 2, 2, NKVC * D], F8,
                          kind="ExternalInput").ap()
    wv_d = nc.dram_tensor("wv", [P, 2, NHT // 2, 2, NKVC * D], F8,
                          kind="ExternalInput").ap()
    wo_d = nc.dram_tensor("wo", [P, NQC, H], FP16, kind="ExternalInput").ap()
    tabs_d = nc.dram_tensor("tabs", [P, 4, S], FP16, kind="ExternalInput").ap()
    msk_d = nc.dram_tensor("msk", [P, 9, P], FP16, kind="ExternalInput").ap()
    y_d = nc.dram_tensor("y", [P, NHT, S], FP16, kind="ExternalOutput").ap()

    with nc.allow_low_precision(reason="fp16/fp8 kernel; rel-err budget 2e-2"), \
         tile.TileContext(nc) as tc:
        with (
            tc.tile_pool(name="const", bufs=1) as cpool,
            tc.tile_pool(name="qkv", bufs=1) as qkv,
            tc.tile_pool(name="wts", bufs=1) as wts,
        ):
            msk_sb = cpool.tile([P, 9, P], FP16, tag="msk")
            ones_sb = cpool.tile([P, P], BF, tag="ones")
            eps_sb = cpool.tile([P, 1], F32, tag="eps")
            eb_sb = cpool.tile([P, 1], F32, tag="eb")
            nc.vector.memset(ones_sb[:], 1.0)
            nc.vector.memset(eps_sb[:], EPS)
            nc.vector.memset(eb_sb[:], EXP_BIAS)
            id_sb = msk_sb[:, 0, :]
            dm_sb = msk_sb[:, 1:3, :]
            em_sb = msk_sb[:, 3:5, :]
            dm01_sb = msk_sb[:, 5:7, :]
            em01_sb = msk_sb[:, 7:9, :]

            wv_sb = wts.tile([P, 2, NHT // 2, 2, NKVC * D], F8, tag="wv")
            wk_sb = wts.tile([P, 2, NHT // 2, 2, NKVC * D], F8, tag="wk")
            wq_sb = wts.tile([P, 2, NHT // 2, 2, NQC * D], F8, tag="wq")
            wo_sb = wts.tile([P, NQC, H], FP16, tag="wo")

            qn_sb = qkv.tile([P, NQC, S], FP16, tag="qn")
            kn_sb = qkv.tile([P, NKVC, S], FP16, tag="kn")
            # x16-scaled fp8 copies, partition-folded (d -> [64, 2] pairs
            # (2p, 2p+1)) for DoubleRow scores on the off-diagonal tiles
            qn8_sb = qkv.tile([64, 2, NQC, S], F8, tag="qn8")
            kn8_sb = qkv.tile([64, 2, NKVC, S], F8, tag="kn8")
            v_sb = qkv.tile([P, NST, NKVC, D + 1], FP16, tag="v")
            nc.vector.memset(v_sb[:, :, :, D:D + 1], 64.0)

            # ---------------- phase 1: QKV projections + RMSNorm + RoPE ----
            with (
                tc.tile_pool(name="hsp", bufs=3) as hsp,
                tc.tile_pool(name="tabp", bufs=3) as tabp,
                tc.tile_pool(name="cpp", bufs=7) as cpp,
                tc.tile_pool(name="t1", bufs=8) as t1p,
                tc.tile_pool(name="t2", bufs=3) as t2p,
                tc.tile_pool(name="t3", bufs=3) as t3p,
                tc.tile_pool(name="t5", bufs=5) as t5p,
                tc.tile_pool(name="t6", bufs=5) as t6p,
                tc.tile_pool(name="pp", bufs=_K("K_PPP", 4), space="PSUM") as ppp,
                tc.tile_pool(name="pvb", bufs=_K("K_PVB", 2), space="PSUM") as pvbp,
                tc.tile_pool(name="psv", bufs=2, space="PSUM") as psvp,
            ):
                # heads order: k0, k1, q0..q3 (wk arrives before wq)
                HEADS = [("k", 0), ("k", 1), ("q", 0), ("q", 1), ("q", 2), ("q", 3)]
                pend = []  # deferred norm/rope finishes (2-deep pipeline)

                def proj_chain(out_ps, w_sb8, mcols, hs_t):
                    first = True
                    for wi, xi in ((0, 0), (0, 1), (1, 0)):
                        for tp in range(NHT // 2):
                            nc.tensor.matmul(
                                out_ps[:], w_sb8[:, wi, tp, :, mcols],
                                hs_t[:, xi, tp, :, :, :],
                                perf_mode=DR, start=first,
                                stop=(wi == 1 and tp == NHT // 2 - 1))
                            first = False

                def finishA(st, tsin_eng=None):
                    # early stage: Pool tsin + DVE square + PE sumsq matmul.
                    # Decoupled from the Pool-gated add so the next head's
                    # sq/vb never queue behind a tsin round-trip.
                    kind, m, pp, cp, u_t, s0, tab_t, rot = st
                    tsin = t1p.tile([P, SCP], FP16, tag="tsin")
                    sin_t = tab_t[:, 1 if kind == "q" else 3, :]
                    eng = nc.gpsimd if _K("K_TSIN", 1) else nc.vector
                    if _K("K_TSINQ3", 0) and s0 == (NCH - 1) * SCP and \
                            kind == "q" and m >= 4 - _K("K_TSINQ3", 0):
                        # last-chunk q tail: Pool's serial tsin chain is the
                        # transition critical path -> run these on DVE where
                        # the in-order queue sequences them before their adds
                        eng = nc.vector
                    (tsin_eng or eng).tensor_mul(tsin[:], rot[:], sin_t)
                    sq = t1p.tile([P, SCP], BF, tag="sq")
                    nc.vector.tensor_mul(sq[:], cp[:], cp[:])
                    vb = pvbp.tile([P, SCP], F32, tag="vb")
                    nc.tensor.matmul(vb[:], ones_sb[:], sq[:], start=True, stop=True)
                    return (kind, m, u_t, s0, tsin, vb)

                def finishB(stB):
                    kind, m, u_t, s0, tsin, vb = stB
                    sd = t2p.tile([P, SCP], FP16, tag="sd")
                    nc.scalar.activation(sd[:], vb[:], AFT.Sqrt, bias=eps_sb[:],
                                         scale=1.0 / D)
                    inv = t3p.tile([P, SCP], FP16, tag="inv")
                    nc.vector.reciprocal(inv[:], sd[:])
                    nc.vector.tensor_add(u_t[:], u_t[:], tsin[:])
                    dst = qn_sb if kind == "q" else kn_sb
                    nc.vector.tensor_mul(dst[:, m, s0:s0 + SCP], u_t[:], inv[:])
                    # fp8 copy + partition-fold for the DR scores path
                    q8u = t2p.tile([P, SCP], F8, tag="q8u")
                    nc.vector.tensor_copy(q8u[:], dst[:, m, s0:s0 + SCP])
                    dst8 = qn8_sb if kind == "q" else kn8_sb
                    nc.scalar.dma_start(out=dst8[:, :, m, s0:s0 + SCP],
                                        in_=q8u[:])

                pend2 = []

                def finish(st, tsin_eng=None):
                    pend2.append(finishA(st, tsin_eng))
                    while len(pend2) > _K("K_PEND2", 1):
                        finishB(pend2.pop(0))

                chunk_tiles = {}

                def load_chunk(sc):
                    s0 = sc * SCP
                    hs_sb = hsp.tile([P, 2, NHT // 2, 2, 4, P], F8, tag="hs",
                                     name="hs_sb")
                    nc.sync.dma_start(out=hs_sb[:, 0], in_=hs_d[:, sc, 0])
                    nc.sync.dma_start(out=hs_sb[:, 1], in_=hs_d[:, sc, 1])
                    tab_sb = tabp.tile([P, 4, SCP], FP16, tag="tab",
                                       name="tab_sb")
                    nc.sync.dma_start(out=tab_sb[:],
                                      in_=tabs_d[:, :, s0:s0 + SCP])
                    chunk_tiles[sc] = (hs_sb, tab_sb)

                for sc in range(NCH):
                    s0 = sc * SCP
                    if sc > 0:
                        hs_sb, tab_sb = chunk_tiles[sc]
                    else:
                        hs_sb = hsp.tile([P, 2, NHT // 2, 2, 4, P], F8,
                                         tag="hs", name="hs_sb")
                        tab_sb = tabp.tile([P, 4, SCP], FP16, tag="tab",
                                           name="tab_sb")
                        # startup: the DMA transfer resource is one serial
                        # FIFO, so everything rides the SP queue in exact
                        # consumption order; hs halves split so the v chain
                        # starts on the first pieces.
                        nc.sync.dma_start(out=wv_sb[:, 0, 0:2], in_=wv_d[:, 0, 0:2])
                        nc.sync.dma_start(out=hs_sb[:, 0, 0:2], in_=hs_d[:, 0, 0, 0:2])
                        nc.sync.dma_start(out=wv_sb[:, 0, 2:8], in_=wv_d[:, 0, 2:8])
                        nc.sync.dma_start(out=hs_sb[:, 0, 2:4], in_=hs_d[:, 0, 0, 2:4])
                        nc.sync.dma_start(out=hs_sb[:, 0, 4:8], in_=hs_d[:, 0, 0, 4:8])
                        nc.sync.dma_start(out=wv_sb[:, 1], in_=wv_d[:, 1])
                        nc.sync.dma_start(out=hs_sb[:, 1, 0:4], in_=hs_d[:, 0, 1, 0:4])
                        nc.sync.dma_start(out=hs_sb[:, 1, 4:8], in_=hs_d[:, 0, 1, 4:8])
                        nc.sync.dma_start(out=wk_sb[:, 0], in_=wk_d[:, 0])
                        nc.sync.dma_start(out=wk_sb[:, 1], in_=wk_d[:, 1])
                        nc.sync.dma_start(out=tab_sb[:], in_=tabs_d[:, :, 0:SCP])
                        nc.sync.dma_start(out=wq_sb[:, 0], in_=wq_d[:, 0])
                        nc.sync.dma_start(out=wq_sb[:, 1], in_=wq_d[:, 1])
                        nc.sync.dma_start(out=msk_sb[:], in_=msk_d[:])

                    # v projection, 2 ss per PSUM tile; chunk 0 runs the three
                    # hi/lo passes ss-interleaved so each input piece is
                    # needed as late as possible.
                    pv_tiles = []
                    for sp in range(2):
                        pv = psvp.tile([P, 2, NKVC * D], F32, tag="pv")
                        pv_tiles.append(pv)
                    # NOTE: matmul accumulation chains must stay contiguous
                    # (interleaving open chains within a PSUM bank corrupts
                    # the accumulation) -> per-ss chains; chunk 0 orders its
                    # passes (hi,hi),(lo-w,hi),(hi,lo-hs) so the late-arriving
                    # hs-lo piece is consumed last.
                    passes = ((0, 0), (1, 0), (0, 1)) if sc == 0 else \
                             ((0, 0), (0, 1), (1, 0))
                    if False:
                        pass
                    else:
                        for ss in range(4):
                            first = True
                            for wi, xi in passes:
                                for tp in range(NHT // 2):
                                    nc.tensor.matmul(
                                        pv_tiles[ss // 2][:, ss % 2, :],
                                        hs_sb[:, xi, tp, :, ss, :],
                                        wv_sb[:, wi, tp, :, :],
                                        perf_mode=DR, start=first,
                                        stop=(wi == 1 and tp == NHT // 2 - 1))
                                    first = False
                    for ss in range(4):
                        st_g = sc * 4 + ss
                        nc.vector.tensor_copy(v_sb[:, st_g, :, 0:D],
                                              pv_tiles[ss // 2][:, ss % 2, :])
                        if ss == 0 and pend:
                            finish(pend.pop(0))
                        if ss == 0 and sc + 1 < NCH:
                            load_chunk(sc + 1)
                        if ss == 0 and sc == 2:
                            nc.sync.dma_start(out=wo_sb[:], in_=wo_d[:])

                    for kind, m in HEADS:
                        w_sb = wq_sb if kind == "q" else wk_sb
                        pp = ppp.tile([P, SCP], F32, tag="pp")
                        proj_chain(pp, w_sb, slice(m * D, (m + 1) * D), hs_sb)
                        cp = cpp.tile([P, SCP], FP16, tag="cp")
                        if _K("K_CPTAIL", 1) and sc == NCH - 1 and \
                                kind == "q" and m >= 2:
                            nc.vector.tensor_copy(cp[:], pp[:])
                        else:
                            nc.scalar.copy(cp[:], pp[:])
                        # rotate_half as a partition-shift DMA (sign and norm
                        # weight are folded into the sin tables on the host);
                        # the two halves ride different HWDGE queues.
                        rot = t6p.tile([P, SCP], FP16, tag="rot")
                        nc.sync.dma_start(out=rot[0:64, :], in_=cp[64:128, :])
                        nc.scalar.dma_start(out=rot[64:128, :], in_=cp[0:64, :])
                        u_t = t5p.tile([P, SCP], FP16, tag="u")
                        cos_t = tab_sb[:, 0 if kind == "q" else 2, :]
                        nc.vector.tensor_mul(u_t[:], cp[:], cos_t)
                        pend.append((kind, m, pp, cp, u_t, s0, tab_sb, rot))
                        depth = 1 if (sc == NCH - 1 and kind == "q" and m >= 1) else 2
                        while len(pend) > depth:
                            finish(pend.pop(0))
                for st in pend:
                    finish(st, tsin_eng=(nc.vector if _K("K_TSINTAIL", 0)
                                         else None))
                while pend2:
                    finishB(pend2.pop(0))
                # preload the Exp act-table set before the attention phase
                dmy = t2p.tile([P, 1], FP16, tag="dmy")
                nc.scalar.activation(dmy[:], eps_sb[:], AFT.Exp)

            # ---------------- phase 2+3: attention + output projection -----
            with (
                tc.tile_pool(name="pb", bufs=_K("K_LAG", 2) + _K("K_PBX", 6)) as pbp,
                tc.tile_pool(name="invp", bufs=4) as invp,
                tc.tile_pool(name="aq", bufs=2) as aqp,
                tc.tile_pool(name="aT", bufs=3) as aTp,
                tc.tile_pool(name="ysb", bufs=2) as ysp,
                tc.tile_pool(name="psc", bufs=_K("K_PSC", 3), space="PSUM") as pscp,
                tc.tile_pool(name="pa", bufs=2, space="PSUM") as pap,
                tc.tile_pool(name="psy", bufs=_K("K_PSY", 3), space="PSUM") as psyp,
            ):
                queue = []
                slab_tiles = {}

                def emit_scores(kvh, u0):
                    # paired unit: both q heads of this kv head at once
                    h0 = 2 * kvh
                    t0 = max(0, u0 - WT)
                    n = u0 - t0 + 1
                    p_t = pbp.tile([P, WT + 1, 2, P], FP16, tag="p", name="p_t")
                    qn_sl = qn_sb[:, h0:h0 + 2, u0 * P:(u0 + 1) * P]
                    qn8_sl = qn8_sb[:, :, h0:h0 + 2, u0 * P:(u0 + 1) * P]
                    for g0 in range(0, n, 2):
                        gn = min(2, n - g0)
                        sc_t = pscp.tile([P, 2, 2, P], F32, tag="sc",
                                         name="sc_t")
                        for i in range(gn):
                            t = t0 + g0 + i
                            # sliding-window mask folded into the PSUM chain
                            # as an additive -60 (exp -> exact fp16 zero)
                            madd = dm_sb if t == u0 else (
                                em_sb if (u0 >= WT and t == u0 - WT) else None)
                            use_mm = _K("K_MASKMM", 1)
                            if t == u0:
                                # diagonal tile in fp16: softmax-peaked rows
                                # live here; fp8 there costs 3x the error
                                nc.tensor.matmul(
                                    sc_t[:, i, :, :],
                                    kn_sb[:, kvh, t * P:(t + 1) * P],
                                    qn_sl, start=True,
                                    stop=(madd is None or not use_mm))
                            else:
                                nc.tensor.matmul(
                                    sc_t[:, i, :, :],
                                    kn8_sb[:, :, kvh, t * P:(t + 1) * P],
                                    qn8_sl, perf_mode=DR, start=True,
                                    stop=(madd is None or not use_mm))
                            if madd is not None and use_mm:
                                nc.tensor.matmul(
                                    sc_t[:, i, :, :], id_sb, madd,
                                    start=False, stop=True)
                        nc.scalar.activation(p_t[:, g0:g0 + gn, :, :],
                                             sc_t[:, 0:gn, :, :], AFT.Exp,
                                             bias=eb_sb[:], scale=1.0 / 256.0)
                        if not _K("K_MASKMM", 1):
                            for i in range(gn):
                                t = t0 + g0 + i
                                if t == u0:
                                    blk = p_t[:, g0 + i, :, :]
                                    nc.gpsimd.tensor_mul(blk, blk, dm01_sb)
                                elif u0 >= WT and t == u0 - WT:
                                    blk = p_t[:, g0 + i, :, :]
                                    nc.gpsimd.tensor_mul(blk, blk, em01_sb)
                    return (kvh, u0, t0, p_t)

                def emit_pv(st):
                    kvh, u0, t0, p_t = st
                    n = u0 - t0 + 1
                    slab = slab_tiles[u0 // 4]
                    for hh in range(2):
                        h = 2 * kvh + hh
                        a_t = pap.tile([P, D + 1], F32, tag="a")
                        for i in range(n):
                            nc.tensor.matmul(
                                a_t[:], p_t[:, i, hh, :],
                                v_sb[:, t0 + i, kvh, :],
                                start=(i == 0), stop=(i == n - 1))
                        inv = invp.tile([P, 1], F32, tag="inv")
                        nc.vector.reciprocal(inv[:], a_t[:, D:D + 1])
                        nc.vector.tensor_scalar_mul(slab[:, h, u0 % 4, :],
                                                    a_t[:, 0:D], inv[:])

                def emit_transpose(s4, h):
                    if ("T", s4) not in slab_tiles:
                        aT = aTp.tile([P, NQC, 4, P], FP16, tag="aT", name="aT")
                        slab_tiles[("T", s4)] = aT
                    aT = slab_tiles[("T", s4)]
                    slab = slab_tiles[s4]
                    eng = nc.scalar if h % 2 == 0 else nc.sync
                    eng.dma_start_transpose(out=aT[:, h, :, :],
                                            in_=slab[:, h, :, :])

                op_queue = []  # (s4, mo) outproj chains, spread across units
                op_state = {}
                ycount = [0]

                def emit_outproj_chain():
                    if not op_queue:
                        return
                    s4, mo = op_queue.pop(0)
                    aT = slab_tiles[("T", s4)]
                    mog, mo4 = divmod(mo, 4)
                    if mo4 == 0:
                        op_state["y"] = ysp.tile([P, 4, SCP], FP16, tag="y",
                                                 name="y_t")
                    y_t = op_state["y"]
                    yp = psyp.tile([P, SCP], F32, tag="yp")
                    for h in range(NQC):
                        nc.tensor.matmul(
                            yp[:], wo_sb[:, h, mo * P:(mo + 1) * P],
                            aT[:, h, :, :],
                            start=(h == 0), stop=(h == NQC - 1))
                    # y copies on DVE (GPSIMD can't read PSUM; Act stays a
                    # pure-exp engine until the tail, where it helps drain)
                    ycount[0] += 1
                    if op_state.get("tail") and ycount[0] % 2 == 0:
                        nc.scalar.copy(y_t[:, mo4, :], yp[:])
                    else:
                        nc.vector.tensor_copy(y_t[:, mo4, :], yp[:])
                    if s4 == 0 or _K("K_YMO", 0):
                        # per-mo output DMAs: finer serial-FIFO granularity
                        nc.sync.dma_start(
                            out=y_d[:, mo, s4 * SCP:(s4 + 1) * SCP],
                            in_=y_t[:, mo4, :])
                    elif mo4 == 3:
                        nc.sync.dma_start(
                            out=y_d[:, mog * 4:(mog + 1) * 4,
                                    s4 * SCP:(s4 + 1) * SCP],
                            in_=y_t[:])

                def emit_outproj(s4):
                    op_queue.extend((s4, mo) for mo in range(NHT))

                # descending u0: the big steady-state units come first and
                # prime the PV pipeline; the small ramp units land at the end
                # where the outproj slabs provide PE filler work.
                # Unit order: prime with kvh=0 big units (q2/q3 rope
                # finishes still in flight), then the tiny ramp slabs 0
                # (their exp latency hides behind big-unit work), then the
                # rest descending so the drain tail lands on slab 1 whose 16
                # outproj chains feed the PE while the last exps trickle in.
                pm = _K("K_PRIMES", 6)
                if pm == 6:
                    UNITS = [(15, 0), (14, 0), (13, 0),
                             (15, 1), (14, 1), (13, 1)]
                elif pm == 7:
                    UNITS = [(15, 0), (14, 0), (13, 0), (12, 0),
                             (11, 0), (10, 0)]
                elif pm == 8:
                    UNITS = [(15, 0), (14, 0), (13, 0), (12, 0),
                             (15, 1), (14, 1)]
                else:
                    UNITS = [(15, 0), (14, 0), (15, 1), (14, 1)]
                big = [(u0, kvh) for u0 in range(13, 3, -1)
                       for kvh in range(NKVC)]
                tiny = [(u0, kvh) for u0 in range(3, -1, -1)
                        for kvh in range(NKVC)]
                big = [u for u in big if u not in UNITS]
                tmode = _K("K_TINY", 1)
                bi = ti = 0
                ratio = {0: 2, 1: 4, 2: 3}[tmode]
                tsave = {0: 0, 1: 4, 2: 2}[tmode]
                while bi < len(big):
                    for _ in range(ratio):
                        if bi < len(big):
                            UNITS.append(big[bi]); bi += 1
                    if ti < len(tiny) - tsave:
                        UNITS.append(tiny[ti]); ti += 1
                UNITS.extend(tiny[ti:])
                slab_pv_left = {}
                for u0, kvh in UNITS:
                    k = (u0 // 4, kvh)
                    slab_pv_left[k] = slab_pv_left.get(k, 0) + 1
                pending_tr = []

                def note_pv_done(u0, kvh):
                    k = (u0 // 4, kvh)
                    slab_pv_left[k] -= 1
                    if slab_pv_left[k] == 0:
                        pending_tr.append(k)

                tr_done = {}

                def emit_slab_transposes(k):
                    s4, kvh = k
                    emit_transpose(s4, 2 * kvh)
                    emit_transpose(s4, 2 * kvh + 1)
                    tr_done[s4] = tr_done.get(s4, 0) + 1
                    if tr_done[s4] == 2:
                        emit_outproj(s4)

                seen_slab = set()
                ucount = [0]
                for u0, kvh in UNITS:
                    if u0 // 4 not in seen_slab:
                        seen_slab.add(u0 // 4)
                        slab_tiles[u0 // 4] = aqp.tile([P, NQC, 4, P], FP16,
                                                       tag="aq", name="aq")
                    queue.append(emit_scores(kvh, u0))
                    ucount[0] += 1
                    lag_eff = _K("K_LAGP", 6) if ucount[0] <= 8 else _K("K_LAG", 2)
                    if len(queue) > lag_eff:
                        st = queue.pop(0)
                        emit_pv(st)
                        note_pv_done(st[1], st[0])
                    if pending_tr:
                        emit_slab_transposes(pending_tr.pop(0))
                    emit_outproj_chain()
                    if len(op_queue) > _K("K_HOLD", 10):
                        emit_outproj_chain()
                # drain: PV tail with transposes interleaved
                while queue:
                    st = queue.pop(0)
                    emit_pv(st)
                    note_pv_done(st[1], st[0])
                    while pending_tr:
                        emit_slab_transposes(pending_tr.pop(0))
                op_state["tail"] = True
                while op_queue:
                    emit_outproj_chain()

    nc.compile()
    _CACHE["nc"] = nc
    return nc


def _host_inputs(hidden_states, wq, wk, wv, wo, q_norm_weight, k_norm_weight):
    """Per-core input dicts (8 cores: c = 4*b + g)."""
    f16 = np.float16
    scale = 1.0 / math.sqrt(D)
    inv_freq = 1.0 / (THETA ** (np.arange(0, D, 2, dtype=np.float64) / D))
    t = np.arange(S, dtype=np.float64)
    freqs = np.outer(t, inv_freq)
    emb = np.concatenate([freqs, freqs], axis=-1)          # [S, D]
    cosT = np.cos(emb).T.astype(np.float64)                # [D, S]
    sinT = np.sin(emb).T.astype(np.float64)
    qw = (1.0 + q_norm_weight).astype(np.float64)
    kw = (1.0 + k_norm_weight).astype(np.float64)

    # rotate_half is done on-device as a plain partition-shift DMA
    # (rot[d] = x[(d+64)%128]); the rotation sign and the SHIFTED norm
    # weight are folded into the sin tables here:
    #   sin_eff[d] = sgn(d) * sin[d] * w[(d+64)%128],  sgn = -1 for d<64
    hh = D // 2
    sgn = np.where(np.arange(D) < hh, -1.0, 1.0)
    qw_sh = np.roll(qw, -hh)   # w[(d+64)%128]
    kw_sh = np.roll(kw, -hh)
    tabs = np.stack([
        cosT * qw[:, None] * scale * 16.0,
        sinT * (sgn * qw_sh)[:, None] * scale * 16.0,
        cosT * kw[:, None] * 16.0,
        sinT * (sgn * kw_sh)[:, None] * 16.0,
    ], axis=1).astype(f16)                                 # [D, 4, S] x16:
    # qn,kn carry x16 so every score tile (fp16 diag and fp8 DR) is x256,
    # absorbed by one exp scale=1/256

    r = np.arange(P)[:, None]
    c = np.arange(P)[None, :]
    dmadd = np.where(c >= r, 0.0, -60.0)                   # [k, q]: allow q >= k
    emadd = np.where(c < r, 0.0, -60.0)                    # [k, q]: allow q < k
    ident = np.eye(P)
    dm01 = np.where(c >= r, 1.0, 0.0)
    em01 = np.where(c < r, 1.0, 0.0)
    msk = np.stack([ident, dmadd, dmadd, emadd, emadd,
                    dm01, dm01, em01, em01],
                   axis=1).astype(f16)                      # [k, 9, q]

    f8 = ml_dtypes.float8_e4m3
    WS = 64.0  # weight scale: fp8 sweet spot; absorbed by RMSNorm (q/k) and
               # by the 64-valued ones-column of v_ext (v)

    def hi_lo(x):
        hi = x.astype(f8)
        lo = (x - hi.astype(np.float64)).astype(f8)
        return np.stack([hi, lo], axis=1)

    def pack_w(w_slice):
        # [O, H] -> lhsT [H, O] -> [128, 2(hl), 8(tp), 2(ti), O] fp8 x64
        wT = w_slice.T.astype(np.float64) * WS
        O = wT.shape[1]
        base = wT.reshape(NHT // 2, 2, P, O).transpose(2, 0, 1, 3)
        return np.ascontiguousarray(hi_lo(base))

    hs_packed = []
    for b in range(B):
        hsT = hidden_states[b].T.astype(np.float64)        # [H, S]
        # [p, sc, 2(hl), 8(tp), 2(ti), 4(q), s128] fp8 hi/lo
        hs6 = hsT.reshape(NHT // 2, 2, P, NCH, 4, P).transpose(2, 3, 0, 1, 4, 5)
        hi = hs6.astype(f8)
        lo = (hs6 - hi.astype(np.float64)).astype(f8)
        hs_packed.append(np.ascontiguousarray(np.stack([hi, lo], axis=2)))

    in_maps = []
    for core in range(8):
        b, g = divmod(core, 4)
        woT = wo[:, 512 * g:512 * (g + 1)].T.astype(np.float64)  # [512, H]
        wo_r = np.ascontiguousarray(
            woT.reshape(NQC, P, H).transpose(1, 0, 2)).astype(f16)
        in_maps.append({
            "hs": hs_packed[b],
            "wq": pack_w(wq[512 * g:512 * (g + 1), :]),
            "wk": pack_w(wk[256 * g:256 * (g + 1), :]),
            "wv": pack_w(wv[256 * g:256 * (g + 1), :]),
            "wo": wo_r,
            "tabs": tabs, "msk": msk,
        })
    return in_maps


def _postprocess(results):
    out = np.empty((B, S, H), np.float32)
    for b in range(B):
        acc = np.zeros((H, S), np.float32)
        for g in range(4):
            y_r = results[4 * b + g]["y"].astype(np.float32)  # [128, 16, S]
            acc += y_r.transpose(1, 0, 2).reshape(H, S)
        out[b] = acc.T
    return out


def kernel(hidden_states, wq, wk, wv, wo, q_norm_weight, k_norm_weight):
    nc = _build_nc()
    in_maps = _host_inputs(hidden_states, wq, wk, wv, wo,
                           q_norm_weight, k_norm_weight)
    res = run_bass_kernel_spmd(nc, in_maps, list(range(8)))
    return _postprocess(res.results)
